# revision 33
# baseline (speedup 1.0000x reference)
"""MeshGNN Trainium2 kernel (fp8 layer 0 + bf16 hidden layers).

Mathematical reduction: the reference broadcasts the text projection to all 12
mesh vertices, and the row-normalized kNN adjacency has uniform row sums, so
node features stay identical across vertices through every GNN layer.  The
network collapses to a per-row MLP; the 12 per-vertex outputs are 12 copies of
the same 3-vector plus the per-vertex template, so the device computes only
the 3-dim displacement and the host broadcasts (exact math, not an
approximation):

    h   = relu(x @ W0c)               W0c = W_text @ (s*W_gnn[0])  (384,256)
    h   = relu(h @ (s*W_gnn[l]))      l = 1..3
    o3  = h @ W_out                   (B, 3)
    out = template[None] + o3[:, None, :]

Precision strategy: layer 0 runs fp8(e4m3) with DoubleRow (2 rows/cycle on
the PE) since x is quantized host-side; hidden activations are bf16 (1-byte
fp8 writes from ACT/DVE run far below rated throughput on real TRN2, so relu
outputs stay 2-byte) and hidden matmuls run bf16 at 1 cycle/row.  Weights are
scaled into each dtype's sweet band and activations rescaled to unit rms in
the relu stage (scale factors from a host-side probe of 512 rows, compensated
exactly in the next layer's weights; final output descale on host in f64).

Device schedule (per core, 4096 rows in 8 blocks of N=512):
  - x shard arrives host-transposed as (384, 4096) fp8; one DMA per block
    (layer 0 runs one DoubleRow pair plus one single k-tile matmul).
  - layer-major software pipeline over all 8 blocks; fused two-bank
    [128,2,512] PSUM tiles (both m-halves, bufs=3) so each relu is one op.
  - relu ops (scale*relu in one instruction) alternate between Activation
    and DVE -- the only engines that can read PSUM on TRN2 (GPSIMD cannot).
  - output layer: block pairs share one [3,2,512] PSUM tile, one SBUF copy
    on ACT or DVE per pair, per-pair DMA of (3, 4096) bf16; host descales
    and broadcasts to (B, 12, 3).
"""

import os

import numpy as np

# ---------------------------------------------------------------- constants
B = 32768
CORES = 8
ROWS = B // CORES            # 4096 rows per core
TD = 384                     # text dim
H = 256                      # hidden
NBLK = 8                     # row blocks per core
N = ROWS // NBLK             # 512 rows per block
MT = H // 128                # 2 m-tiles for hidden outputs

RELU_MODE = os.environ.get("MESHGNN_RELU", "fused")   # "fused" | "half"
PAIR = os.environ.get("MESHGNN_PAIR", "0") == "1"     # stationary-reuse pair issue
HDT = os.environ.get("MESHGNN_HDT", "bf16")           # hidden act dtype: "bf16" | "f32"
NOSCALE = os.environ.get("MESHGNN_NOSCALE", "0") == "1"  # fold relu scales into weights

# per-op engine cost estimates (ns) for the load-greedy relu/copy schedule
if RELU_MODE == "fused":
    ENG_COST_RELU = {"A": 1118.0, "D": 1352.0}
else:
    ENG_COST_RELU = {"A": 692.0, "D": 738.0}
# copies forced to DVE: ACT then only ever runs Relu, avoiding any
# HW-side activation-table thrash between Identity and Relu (the table
# reload insertion happens in neuronxcc lower_act, which CoreSim does not
# model)
ENG_COST_COPY = {"A": 1e9, "D": 730.0}

_BUILT = {}                  # cache: compiled Bass modules + fold results


def _schedule_engines():
    """Greedy engine assignment: relus[l][b][m] and copies[b] (ACT/DVE only).

    In "fused" mode both m-halves share one op; relus[l][b][0] is used.
    """
    load = {"A": 1283.0, "D": 0.0}  # act-table preload
    relus = [[[None] * MT for _ in range(NBLK)] for _ in range(4)]
    copies = [None] * (NBLK // 2)
    nm = 1 if RELU_MODE == "fused" else MT
    for l in range(4):
        for b in range(NBLK):
            for m in range(nm):
                e = min("AD", key=lambda k: load[k] + ENG_COST_RELU[k])
                load[e] += ENG_COST_RELU[e]
                relus[l][b][m] = e
            if l == 3 and b % 2 == 1:
                e = min("AD", key=lambda k: load[k] + ENG_COST_COPY[k])
                load[e] += ENG_COST_COPY[e]
                copies[b // 2] = e
    return relus, copies


def _build_bass(repeat=1, fake_relu=False, loop_repeat=0, zero_bias=None):
    """Build + compile the per-core Bass program (same NEFF on all cores).

    loop_repeat > 0 wraps the pipeline in a device-side For_i loop executed
    that many times (identical outputs) -- used for dispatch-free HW timing.
    """
    import concourse.mybir as mybir
    import concourse.tile as tile
    from concourse import bacc

    cl = _BUILT["act_scales"]          # [c0..c3] set by _make_in_maps

    f32 = mybir.dt.float32
    bf16 = mybir.dt.bfloat16
    fp8 = mybir.dt.float8e4
    hdt = mybir.dt.float32r if HDT == "f32" else bf16
    DR = mybir.MatmulPerfMode.DoubleRow
    RELU = mybir.ActivationFunctionType.Relu
    IDENT = mybir.ActivationFunctionType.Identity
    MAX = mybir.AluOpType.max
    MULT = mybir.AluOpType.mult

    RELU_ENG, COPY_ENG = _schedule_engines()

    nc = bacc.Bacc(
        "TRN2",
        target_bir_lowering=False,
        debug=False,
        enable_asserts=False,
        num_devices=CORES,
    )

    xt_d = nc.dram_tensor("xt", (TD, ROWS), fp8, kind="ExternalInput")
    w0_d = nc.dram_tensor("w0", (TD, H), fp8, kind="ExternalInput")
    wh_d = nc.dram_tensor("wh", (3 * H, H), hdt, kind="ExternalInput")
    w4_d = nc.dram_tensor("w4", (H, 3), hdt, kind="ExternalInput")
    out_d = nc.dram_tensor("out", (3, ROWS), bf16, kind="ExternalOutput")

    # x viewed as (partition, ktile, row)
    xt_v = xt_d.ap().rearrange("(k p) n -> p k n", p=128)

    with tile.TileContext(nc) as tc:
        with (
            tc.tile_pool(name="wp", bufs=1) as wp,
            tc.tile_pool(name="xp", bufs=6) as xp,
            tc.tile_pool(
                name="hp", bufs=(8 if HDT == "bf16" else 2)
            ) as hp,
            tc.tile_pool(name="ob", bufs=2) as obp,
            tc.tile_pool(
                name="pp", bufs=(3 if RELU_MODE == "fused" else 6),
                space="PSUM",
            ) as pp,
            tc.tile_pool(name="pp4", bufs=1, space="PSUM") as pp4,
        ):
            # ---- weights, loaded once (w0 first on SP so layer 0 starts
            # ~1.5us in; the one-time act-table load hides in the fill)
            # w0: [128, 3, 256]; stationary APs: DR pair + single k-tile
            w0_t = wp.tile([128, 3, H], fp8, tag="w0")
            nc.sync.dma_start(
                w0_t[:], w0_d.ap().rearrange("(i p) m -> p i m", p=128)
            )
            # hidden weights: [128, 6, 256]; layer l (1..3) ktile j=(l-1)*2
            wh_t = wp.tile([128, 6, H], hdt, tag="wh")
            nc.gpsimd.dma_start(
                wh_t[:], wh_d.ap().rearrange("(li p) m -> p li m", p=128)
            )
            # output weights: [128, 2, 3]
            w4_t = wp.tile([128, 2, 3], hdt, tag="w4")
            nc.scalar.dma_start(
                w4_t[:], w4_d.ap().rearrange("(i p) m -> p i m", p=128)
            )

            fk_t = None
            if fake_relu:
                fk_t = wp.tile([128, MT, N], f32, tag="fk")
                nc.gpsimd.memset(fk_t[:], 1.0)

            def relu_op(eng, dst, src, c):
                if fake_relu:
                    src = fk_t[:]
                if eng == "A":
                    if c == 1.0:
                        nc.scalar.activation(dst, src, RELU)
                    else:
                        nc.scalar.activation(dst, src, RELU, scale=c)
                elif c == 1.0:
                    nc.vector.tensor_scalar(dst, src, 0.0, None, MAX)
                else:
                    nc.vector.tensor_scalar(dst, src, c, 0.0, MULT, MAX)

            def copy_op(eng, dst, src):
                if eng == "A":
                    nc.scalar.activation(dst, src, IDENT)
                else:
                    nc.vector.tensor_scalar(dst, src, 1.0, None, MULT)

            import contextlib

            loop_cm = (
                tc.For_i(0, loop_repeat, 1) if loop_repeat
                else contextlib.nullcontext()
            )
            with loop_cm:
                for rep in range(repeat):
                    xts = {}
                    for b in range(NBLK):
                        xt = xp.tile([128, 3, N], fp8, tag=f"x{b % 6}")
                        nc.sync.dma_start(xt[:], xt_v[:, :, b * N:(b + 1) * N])
                        xts[b] = xt

                    h_prev = {}
                    if PAIR:
                        # pair-issue: each stationary weight tile feeds two
                        # consecutive blocks' matmuls back-to-back
                        for l in range(4):
                            for bp in range(0, NBLK, 2):
                                bs = (bp, bp + 1)
                                hcs = {
                                    b: hp.tile([128, MT, N], hdt,
                                               name=f"h{l}{b}",
                                               tag=f"h{b % 8}")
                                    for b in bs
                                }
                                pss = {
                                    b: pp.tile([128, MT, N], f32,
                                               name="psb", tag="ps")
                                    for b in bs
                                }
                                for m in range(MT):
                                    ms = slice(m * 128, (m + 1) * 128)
                                    if l == 0:
                                        for b in bs:
                                            nc.tensor.matmul(
                                                pss[b][:, m, :],
                                                w0_t[:, 0:2, ms],
                                                xts[b][:, 0:2, :],
                                                start=True, stop=False,
                                                perf_mode=DR,
                                            )
                                        for b in bs:
                                            nc.tensor.matmul(
                                                pss[b][:, m, :],
                                                w0_t[:, 2, ms],
                                                xts[b][:, 2, :],
                                                start=False, stop=True,
                                            )
                                    else:
                                        j = (l - 1) * 2
                                        for k in range(MT):
                                            for b in bs:
                                                nc.tensor.matmul(
                                                    pss[b][:, m, :],
                                                    wh_t[:, j + k, ms],
                                                    h_prev[b][:, k, :],
                                                    start=(k == 0),
                                                    stop=(k == MT - 1),
                                                )
                                for b in bs:
                                    relu_op(
                                        RELU_ENG[l][b][0],
                                        hcs[b][:].rearrange("p a b -> p (a b)"),
                                        pss[b][:].rearrange("p a b -> p (a b)"),
                                        cl[l],
                                    )
                                    h_prev[b] = hcs[b]
                                if l == 3:
                                    ps4 = pp4.tile([3, 2, N], f32, tag="ps4")
                                    for i, b in enumerate(bs):
                                        for k in range(MT):
                                            nc.tensor.matmul(
                                                ps4[:, i, :], w4_t[:, k, :],
                                                h_prev[b][:, k, :],
                                                start=(k == 0),
                                                stop=(k == MT - 1),
                                            )
                                    ob = obp.tile([3, 2, N], bf16, tag="ob")
                                    copy_op(COPY_ENG[bp // 2], ob[:], ps4[:])
                                    nc.sync.dma_start(
                                        out_d.ap()[:, bp * N:(bp + 2) * N],
                                        ob[:].rearrange("p a b -> p (a b)"),
                                    )
                    for l in range(4) if not PAIR else ():
                        for b in range(NBLK):
                            hc = hp.tile(
                                [128, MT, N], hdt,
                                name=f"h{l}{b}", tag=f"h{b % 8}",
                            )
                            fused = RELU_MODE == "fused"
                            ps = pp.tile(
                                [128, MT, N] if fused else [128, N],
                                f32, name="psb", tag="ps",
                            )
                            pss = {}
                            for m in range(MT):
                                ms = slice(m * 128, (m + 1) * 128)
                                pm = ps[:, m, :] if fused else ps[:]
                                if not fused and m > 0:
                                    ps = pp.tile([128, N], f32,
                                                 name="psb", tag="ps")
                                    pm = ps[:]
                                pss[m] = (ps, pm)
                                if l == 0:
                                    nc.tensor.matmul(
                                        pm, w0_t[:, 0:2, ms],
                                        xts[b][:, 0:2, :],
                                        start=True, stop=False, perf_mode=DR,
                                    )
                                    nc.tensor.matmul(
                                        pm, w0_t[:, 2, ms],
                                        xts[b][:, 2, :],
                                        start=False, stop=True,
                                    )
                                else:
                                    j = (l - 1) * 2
                                    for k in range(MT):
                                        nc.tensor.matmul(
                                            pm, wh_t[:, j + k, ms],
                                            h_prev[b][:, k, :],
                                            start=(k == 0), stop=(k == MT - 1),
                                        )
                                if not fused:
                                    relu_op(RELU_ENG[l][b][m],
                                            hc[:, m, :], pm, cl[l])
                            if fused:
                                # 2D flattened APs: same contiguous bytes,
                                # avoids the strided-3D slow path
                                relu_op(
                                    RELU_ENG[l][b][0],
                                    hc[:].rearrange("p a b -> p (a b)"),
                                    ps[:].rearrange("p a b -> p (a b)"),
                                    cl[l],
                                )
                            h_prev[b] = hc
                            if l == 3:
                                # output layer: block pairs share a psum tile
                                if b % 2 == 0:
                                    ps4 = pp4.tile([3, 2, N], f32, tag="ps4")
                                    last_ps4 = ps4
                                else:
                                    ps4 = last_ps4
                                for k in range(MT):
                                    nc.tensor.matmul(
                                        ps4[:, b % 2, :], w4_t[:, k, :],
                                        hc[:, k, :],
                                        start=(k == 0), stop=(k == MT - 1),
                                    )
                                if b % 2 == 1:
                                    ob = obp.tile([3, 2, N], bf16, tag="ob")
                                    copy_op(COPY_ENG[b // 2], ob[:], ps4[:])
                                    nc.sync.dma_start(
                                        out_d.ap()[
                                            :, (b - 1) * N:(b + 1) * N
                                        ],
                                        ob[:].rearrange("p a b -> p (a b)"),
                                    )

    nc.compile()
    return nc


def _fold_weights(x, W_text, b_text, W_gnn, b_gnn, W_out, b_out, adjacency,
                  template):
    """Fold the GNN into a 5-matrix MLP, compute fp8 scale chain from a probe."""
    s_rows = adjacency.astype(np.float64).sum(axis=1)
    if np.ptp(s_rows) > 1e-5:
        raise ValueError("adjacency row sums are not uniform; collapse invalid")
    s = float(s_rows.mean())
    if not (np.all(b_text == 0) and np.all(b_gnn == 0) and np.all(b_out == 0)):
        raise ValueError("nonzero biases unsupported by fp8 kernel")

    W0c = W_text.astype(np.float64) @ (s * W_gnn[0].astype(np.float64))
    Wl = [s * W_gnn[l].astype(np.float64) for l in (1, 2, 3)]
    W4 = W_out.astype(np.float64)

    # probe the true network to get per-layer rms statistics
    xp = x[:512].astype(np.float64)
    z = xp @ W0c
    gamma = []           # 1/rms(h_l)
    h = np.maximum(z, 0.0)
    gamma.append(1.0 / np.sqrt((h ** 2).mean()))
    for l in range(3):
        z = h @ Wl[l]
        h = np.maximum(z, 0.0)
        gamma.append(1.0 / np.sqrt((h ** 2).mean()))

    import concourse.mybir as mybir
    np8 = mybir.dt.np(mybir.dt.float8e4)
    npb = mybir.dt.np(mybir.dt.bfloat16)

    nph = np.float32 if HDT == "f32" else npb

    def centered_q(Wb, dt=np8):
        u = 2.0 ** round(np.log2(4.0 / Wb.std()))
        return np.ascontiguousarray((Wb * u).astype(np.float32)).astype(dt), u

    W0q, u0 = centered_q(W0c)
    Whq = []
    if NOSCALE:
        # pure max() relus: weight-centering scales accumulate through the
        # layers (bf16 range is plenty) and divide out once on host
        act_scales = [1.0, 1.0, 1.0, 1.0]
        s = u0
        for l in range(3):
            Wq, u = centered_q(Wl[l], nph)
            Whq.append(Wq)
            s *= u
        W4q, u4 = centered_q(W4, nph)
        descale = 1.0 / (s * u4)
    else:
        act_scales = [gamma[0] / u0]
        for l in range(3):
            Wq, u = centered_q(Wl[l] / gamma[l], nph)
            Whq.append(Wq)
            act_scales.append(gamma[l + 1] / u)
        W4q, u4 = centered_q(W4 / gamma[3], nph)
        descale = 1.0 / u4

    return {
        "w0": W0q,
        "wh": np.ascontiguousarray(np.concatenate(Whq, axis=0)),
        "w4": W4q,
        "act_scales": [float(c) for c in act_scales],
        "out_descale": float(descale),
    }


def _make_in_maps(inputs):
    x = np.asarray(inputs["text_emb"], dtype=np.float32)
    fold = _fold_weights(
        x, np.asarray(inputs["W_text"]), np.asarray(inputs["b_text"]),
        np.asarray(inputs["W_gnn"]), np.asarray(inputs["b_gnn"]),
        np.asarray(inputs["W_out"]), np.asarray(inputs["b_out"]),
        np.asarray(inputs["adjacency"]), np.asarray(inputs["template"]),
    )
    _BUILT.setdefault("act_scales", fold["act_scales"])
    _BUILT.setdefault("out_descale", fold["out_descale"])
    _BUILT.setdefault("template", np.asarray(inputs["template"], np.float32))

    import concourse.mybir as mybir
    np8 = mybir.dt.np(mybir.dt.float8e4)
    in_maps = []
    for c in range(CORES):
        shard = np.ascontiguousarray(
            x[c * ROWS:(c + 1) * ROWS].T
        ).astype(np8)
        in_maps.append({
            "xt": shard, "w0": fold["w0"], "wh": fold["wh"], "w4": fold["w4"],
        })
    return in_maps


def kernel(**inputs):
    from concourse.bass_utils import run_bass_kernel_spmd

    in_maps = _make_in_maps(inputs)
    if "nc" not in _BUILT:
        _BUILT["nc"] = _build_bass(repeat=1)
    nc = _BUILT["nc"]
    res = run_bass_kernel_spmd(nc, in_maps, core_ids=list(range(CORES)))
    _BUILT["last_results"] = res
    _BUILT["last_in_maps"] = in_maps

    o3 = np.empty((B, 3), dtype=np.float64)
    for c in range(CORES):
        o3[c * ROWS:(c + 1) * ROWS] = res.results[c]["out"].astype(np.float64).T
    o3 *= _BUILT["out_descale"]
    out = (
        _BUILT["template"][None, :, :].astype(np.float64)
        + o3[:, None, :]
    ).astype(np.float32)
    return out


# revision 34
# speedup vs baseline: 1.3875x; 1.3875x over previous
"""MeshGNN Trainium2 kernel (fp8 layer 0 + bf16 hidden layers).

Mathematical reduction: the reference broadcasts the text projection to all 12
mesh vertices, and the row-normalized kNN adjacency has uniform row sums, so
node features stay identical across vertices through every GNN layer.  The
network collapses to a per-row MLP; the 12 per-vertex outputs are 12 copies of
the same 3-vector plus the per-vertex template, so the device computes only
the 3-dim displacement and the host broadcasts (exact math, not an
approximation):

    h   = relu(x @ W0c)               W0c = W_text @ (s*W_gnn[0])  (384,256)
    h   = relu(h @ (s*W_gnn[l]))      l = 1..3
    o3  = h @ W_out                   (B, 3)
    out = template[None] + o3[:, None, :]

Precision strategy: layer 0 runs fp8(e4m3) with DoubleRow (2 rows/cycle on
the PE) since x is quantized host-side; hidden activations are bf16 (1-byte
fp8 writes from ACT/DVE run far below rated throughput on real TRN2, so relu
outputs stay 2-byte) and hidden matmuls run bf16 at 1 cycle/row.  Weights are
scaled into each dtype's sweet band and activations rescaled to unit rms in
the relu stage (scale factors from a host-side probe of 512 rows, compensated
exactly in the next layer's weights; final output descale on host in f64).

Device schedule (per core, 4096 rows in 8 blocks of N=512):
  - x shard arrives host-transposed as (384, 4096) fp8; one DMA per block
    (layer 0 runs one DoubleRow pair plus one single k-tile matmul).
  - layer-major software pipeline over all 8 blocks; fused two-bank
    [128,2,512] PSUM tiles (both m-halves, bufs=3) so each relu is one op.
  - relu ops (scale*relu in one instruction) alternate between Activation
    and DVE -- the only engines that can read PSUM on TRN2 (GPSIMD cannot).
  - output layer: block pairs share one [3,2,512] PSUM tile, one SBUF copy
    on ACT or DVE per pair, per-pair DMA of (3, 4096) bf16; host descales
    and broadcasts to (B, 12, 3).
"""

import os

import numpy as np

# ---------------------------------------------------------------- constants
B = 32768
CORES = 8
ROWS = B // CORES            # 4096 rows per core
TD = 384                     # text dim
H = 256                      # hidden
NBLK = 8                     # row blocks per core
N = ROWS // NBLK             # 512 rows per block
MT = H // 128                # 2 m-tiles for hidden outputs

RELU_MODE = os.environ.get("MESHGNN_RELU", "fused")   # "fused" | "half"
PAIR = os.environ.get("MESHGNN_PAIR", "0") == "1"     # stationary-reuse pair issue
HDT = os.environ.get("MESHGNN_HDT", "bf16")           # hidden act dtype: "bf16" | "f32"
NOSCALE = os.environ.get("MESHGNN_NOSCALE", "0") == "1"  # fold relu scales into weights

# per-op engine cost estimates (ns) for the load-greedy relu/copy schedule
if RELU_MODE == "fused":
    ENG_COST_RELU = {"A": 1118.0, "D": 1352.0}
else:
    ENG_COST_RELU = {"A": 692.0, "D": 738.0}
ENG_COST_COPY = {"A": 670.0, "D": 730.0}

_BUILT = {}                  # cache: compiled Bass modules + fold results


def _schedule_engines():
    """Greedy engine assignment: relus[l][b][m] and copies[b] (ACT/DVE only).

    In "fused" mode both m-halves share one op; relus[l][b][0] is used.
    """
    load = {"A": 1283.0, "D": 0.0}  # act-table preload
    relus = [[[None] * MT for _ in range(NBLK)] for _ in range(4)]
    copies = [None] * (NBLK // 2)
    nm = 1 if RELU_MODE == "fused" else MT
    for l in range(4):
        for b in range(NBLK):
            for m in range(nm):
                e = min("AD", key=lambda k: load[k] + ENG_COST_RELU[k])
                load[e] += ENG_COST_RELU[e]
                relus[l][b][m] = e
            if l == 3 and b % 2 == 1:
                e = min("AD", key=lambda k: load[k] + ENG_COST_COPY[k])
                load[e] += ENG_COST_COPY[e]
                copies[b // 2] = e
    return relus, copies


def _build_bass(repeat=1, fake_relu=False, loop_repeat=0, zero_bias=None):
    """Build + compile the per-core Bass program (same NEFF on all cores).

    loop_repeat > 0 wraps the pipeline in a device-side For_i loop executed
    that many times (identical outputs) -- used for dispatch-free HW timing.
    """
    import concourse.mybir as mybir
    import concourse.tile as tile
    from concourse import bacc

    cl = _BUILT["act_scales"]          # [c0..c3] set by _make_in_maps

    f32 = mybir.dt.float32
    bf16 = mybir.dt.bfloat16
    fp8 = mybir.dt.float8e4
    hdt = mybir.dt.float32r if HDT == "f32" else bf16
    DR = mybir.MatmulPerfMode.DoubleRow
    RELU = mybir.ActivationFunctionType.Relu
    IDENT = mybir.ActivationFunctionType.Identity
    MAX = mybir.AluOpType.max
    MULT = mybir.AluOpType.mult

    RELU_ENG, COPY_ENG = _schedule_engines()

    nc = bacc.Bacc(
        "TRN2",
        target_bir_lowering=False,
        debug=False,
        enable_asserts=False,
        num_devices=CORES,
    )

    xt_d = nc.dram_tensor("xt", (TD, ROWS), fp8, kind="ExternalInput")
    w0_d = nc.dram_tensor("w0", (TD, H), fp8, kind="ExternalInput")
    wh_d = nc.dram_tensor("wh", (3 * H, H), hdt, kind="ExternalInput")
    w4_d = nc.dram_tensor("w4", (H, 3), hdt, kind="ExternalInput")
    out_d = nc.dram_tensor("out", (3, ROWS), bf16, kind="ExternalOutput")

    # x viewed as (partition, ktile, row)
    xt_v = xt_d.ap().rearrange("(k p) n -> p k n", p=128)

    with tile.TileContext(nc) as tc:
        with (
            tc.tile_pool(name="wp", bufs=1) as wp,
            tc.tile_pool(name="xp", bufs=6) as xp,
            tc.tile_pool(
                name="hp", bufs=(8 if HDT == "bf16" else 2)
            ) as hp,
            tc.tile_pool(name="ob", bufs=2) as obp,
            tc.tile_pool(
                name="pp", bufs=(3 if RELU_MODE == "fused" else 6),
                space="PSUM",
            ) as pp,
            tc.tile_pool(name="pp4", bufs=1, space="PSUM") as pp4,
        ):
            # ---- weights, loaded once (w0 first on SP so layer 0 starts
            # ~1.5us in; the one-time act-table load hides in the fill)
            # w0: [128, 3, 256]; stationary APs: DR pair + single k-tile
            w0_t = wp.tile([128, 3, H], fp8, tag="w0")
            nc.sync.dma_start(
                w0_t[:], w0_d.ap().rearrange("(i p) m -> p i m", p=128)
            )
            # hidden weights: [128, 6, 256]; layer l (1..3) ktile j=(l-1)*2
            wh_t = wp.tile([128, 6, H], hdt, tag="wh")
            nc.gpsimd.dma_start(
                wh_t[:], wh_d.ap().rearrange("(li p) m -> p li m", p=128)
            )
            # output weights: [128, 2, 3]
            w4_t = wp.tile([128, 2, 3], hdt, tag="w4")
            nc.scalar.dma_start(
                w4_t[:], w4_d.ap().rearrange("(i p) m -> p i m", p=128)
            )

            fk_t = None
            if fake_relu:
                fk_t = wp.tile([128, MT, N], f32, tag="fk")
                nc.gpsimd.memset(fk_t[:], 1.0)

            def relu_op(eng, dst, src, c):
                if fake_relu:
                    src = fk_t[:]
                if eng == "A":
                    if c == 1.0:
                        nc.scalar.activation(dst, src, RELU)
                    else:
                        nc.scalar.activation(dst, src, RELU, scale=c)
                elif c == 1.0:
                    nc.vector.tensor_scalar(dst, src, 0.0, None, MAX)
                else:
                    nc.vector.tensor_scalar(dst, src, c, 0.0, MULT, MAX)

            def copy_op(eng, dst, src):
                if eng == "A":
                    nc.scalar.activation(dst, src, IDENT)
                else:
                    nc.vector.tensor_scalar(dst, src, 1.0, None, MULT)

            import contextlib

            loop_cm = (
                tc.For_i(0, loop_repeat, 1) if loop_repeat
                else contextlib.nullcontext()
            )
            with loop_cm:
                for rep in range(repeat):
                    xts = {}
                    for b in range(NBLK):
                        xt = xp.tile([128, 3, N], fp8, tag=f"x{b % 6}")
                        nc.sync.dma_start(xt[:], xt_v[:, :, b * N:(b + 1) * N])
                        xts[b] = xt

                    h_prev = {}
                    if PAIR:
                        # pair-issue: each stationary weight tile feeds two
                        # consecutive blocks' matmuls back-to-back
                        for l in range(4):
                            for bp in range(0, NBLK, 2):
                                bs = (bp, bp + 1)
                                hcs = {
                                    b: hp.tile([128, MT, N], hdt,
                                               name=f"h{l}{b}",
                                               tag=f"h{b % 8}")
                                    for b in bs
                                }
                                pss = {
                                    b: pp.tile([128, MT, N], f32,
                                               name="psb", tag="ps")
                                    for b in bs
                                }
                                for m in range(MT):
                                    ms = slice(m * 128, (m + 1) * 128)
                                    if l == 0:
                                        for b in bs:
                                            nc.tensor.matmul(
                                                pss[b][:, m, :],
                                                w0_t[:, 0:2, ms],
                                                xts[b][:, 0:2, :],
                                                start=True, stop=False,
                                                perf_mode=DR,
                                            )
                                        for b in bs:
                                            nc.tensor.matmul(
                                                pss[b][:, m, :],
                                                w0_t[:, 2, ms],
                                                xts[b][:, 2, :],
                                                start=False, stop=True,
                                            )
                                    else:
                                        j = (l - 1) * 2
                                        for k in range(MT):
                                            for b in bs:
                                                nc.tensor.matmul(
                                                    pss[b][:, m, :],
                                                    wh_t[:, j + k, ms],
                                                    h_prev[b][:, k, :],
                                                    start=(k == 0),
                                                    stop=(k == MT - 1),
                                                )
                                for b in bs:
                                    relu_op(
                                        RELU_ENG[l][b][0],
                                        hcs[b][:].rearrange("p a b -> p (a b)"),
                                        pss[b][:].rearrange("p a b -> p (a b)"),
                                        cl[l],
                                    )
                                    h_prev[b] = hcs[b]
                                if l == 3:
                                    ps4 = pp4.tile([3, 2, N], f32, tag="ps4")
                                    for i, b in enumerate(bs):
                                        for k in range(MT):
                                            nc.tensor.matmul(
                                                ps4[:, i, :], w4_t[:, k, :],
                                                h_prev[b][:, k, :],
                                                start=(k == 0),
                                                stop=(k == MT - 1),
                                            )
                                    ob = obp.tile([3, 2, N], bf16, tag="ob")
                                    copy_op(COPY_ENG[bp // 2], ob[:], ps4[:])
                                    nc.sync.dma_start(
                                        out_d.ap()[:, bp * N:(bp + 2) * N],
                                        ob[:].rearrange("p a b -> p (a b)"),
                                    )
                    for l in range(4) if not PAIR else ():
                        for b in range(NBLK):
                            hc = hp.tile(
                                [128, MT, N], hdt,
                                name=f"h{l}{b}", tag=f"h{b % 8}",
                            )
                            fused = RELU_MODE == "fused"
                            ps = pp.tile(
                                [128, MT, N] if fused else [128, N],
                                f32, name="psb", tag="ps",
                            )
                            pss = {}
                            for m in range(MT):
                                ms = slice(m * 128, (m + 1) * 128)
                                pm = ps[:, m, :] if fused else ps[:]
                                if not fused and m > 0:
                                    ps = pp.tile([128, N], f32,
                                                 name="psb", tag="ps")
                                    pm = ps[:]
                                pss[m] = (ps, pm)
                                if l == 0:
                                    nc.tensor.matmul(
                                        pm, w0_t[:, 0:2, ms],
                                        xts[b][:, 0:2, :],
                                        start=True, stop=False, perf_mode=DR,
                                    )
                                    nc.tensor.matmul(
                                        pm, w0_t[:, 2, ms],
                                        xts[b][:, 2, :],
                                        start=False, stop=True,
                                    )
                                else:
                                    j = (l - 1) * 2
                                    for k in range(MT):
                                        nc.tensor.matmul(
                                            pm, wh_t[:, j + k, ms],
                                            h_prev[b][:, k, :],
                                            start=(k == 0), stop=(k == MT - 1),
                                        )
                                if not fused:
                                    relu_op(RELU_ENG[l][b][m],
                                            hc[:, m, :], pm, cl[l])
                            if fused:
                                # 2D flattened APs: same contiguous bytes,
                                # avoids the strided-3D slow path
                                relu_op(
                                    RELU_ENG[l][b][0],
                                    hc[:].rearrange("p a b -> p (a b)"),
                                    ps[:].rearrange("p a b -> p (a b)"),
                                    cl[l],
                                )
                            h_prev[b] = hc
                            if l == 3:
                                # output layer: block pairs share a psum tile
                                if b % 2 == 0:
                                    ps4 = pp4.tile([3, 2, N], f32, tag="ps4")
                                    last_ps4 = ps4
                                else:
                                    ps4 = last_ps4
                                for k in range(MT):
                                    nc.tensor.matmul(
                                        ps4[:, b % 2, :], w4_t[:, k, :],
                                        hc[:, k, :],
                                        start=(k == 0), stop=(k == MT - 1),
                                    )
                                if b % 2 == 1:
                                    ob = obp.tile([3, 2, N], bf16, tag="ob")
                                    copy_op(COPY_ENG[b // 2], ob[:], ps4[:])
                                    nc.sync.dma_start(
                                        out_d.ap()[
                                            :, (b - 1) * N:(b + 1) * N
                                        ],
                                        ob[:].rearrange("p a b -> p (a b)"),
                                    )

    nc.compile()
    return nc


def _fold_weights(x, W_text, b_text, W_gnn, b_gnn, W_out, b_out, adjacency,
                  template):
    """Fold the GNN into a 5-matrix MLP, compute fp8 scale chain from a probe."""
    s_rows = adjacency.astype(np.float64).sum(axis=1)
    if np.ptp(s_rows) > 1e-5:
        raise ValueError("adjacency row sums are not uniform; collapse invalid")
    s = float(s_rows.mean())
    if not (np.all(b_text == 0) and np.all(b_gnn == 0) and np.all(b_out == 0)):
        raise ValueError("nonzero biases unsupported by fp8 kernel")

    W0c = W_text.astype(np.float64) @ (s * W_gnn[0].astype(np.float64))
    Wl = [s * W_gnn[l].astype(np.float64) for l in (1, 2, 3)]
    W4 = W_out.astype(np.float64)

    # probe the true network to get per-layer rms statistics
    xp = x[:512].astype(np.float64)
    z = xp @ W0c
    gamma = []           # 1/rms(h_l)
    h = np.maximum(z, 0.0)
    gamma.append(1.0 / np.sqrt((h ** 2).mean()))
    for l in range(3):
        z = h @ Wl[l]
        h = np.maximum(z, 0.0)
        gamma.append(1.0 / np.sqrt((h ** 2).mean()))

    import concourse.mybir as mybir
    np8 = mybir.dt.np(mybir.dt.float8e4)
    npb = mybir.dt.np(mybir.dt.bfloat16)

    nph = np.float32 if HDT == "f32" else npb

    def centered_q(Wb, dt=np8):
        u = 2.0 ** round(np.log2(4.0 / Wb.std()))
        return np.ascontiguousarray((Wb * u).astype(np.float32)).astype(dt), u

    W0q, u0 = centered_q(W0c)
    Whq = []
    if NOSCALE:
        # pure max() relus: weight-centering scales accumulate through the
        # layers (bf16 range is plenty) and divide out once on host
        act_scales = [1.0, 1.0, 1.0, 1.0]
        s = u0
        for l in range(3):
            Wq, u = centered_q(Wl[l], nph)
            Whq.append(Wq)
            s *= u
        W4q, u4 = centered_q(W4, nph)
        descale = 1.0 / (s * u4)
    else:
        act_scales = [gamma[0] / u0]
        for l in range(3):
            Wq, u = centered_q(Wl[l] / gamma[l], nph)
            Whq.append(Wq)
            act_scales.append(gamma[l + 1] / u)
        W4q, u4 = centered_q(W4 / gamma[3], nph)
        descale = 1.0 / u4

    return {
        "w0": W0q,
        "wh": np.ascontiguousarray(np.concatenate(Whq, axis=0)),
        "w4": W4q,
        "act_scales": [float(c) for c in act_scales],
        "out_descale": float(descale),
    }


def _make_in_maps(inputs):
    x = np.asarray(inputs["text_emb"], dtype=np.float32)
    fold = _fold_weights(
        x, np.asarray(inputs["W_text"]), np.asarray(inputs["b_text"]),
        np.asarray(inputs["W_gnn"]), np.asarray(inputs["b_gnn"]),
        np.asarray(inputs["W_out"]), np.asarray(inputs["b_out"]),
        np.asarray(inputs["adjacency"]), np.asarray(inputs["template"]),
    )
    _BUILT.setdefault("act_scales", fold["act_scales"])
    _BUILT.setdefault("out_descale", fold["out_descale"])
    _BUILT.setdefault("template", np.asarray(inputs["template"], np.float32))

    import concourse.mybir as mybir
    np8 = mybir.dt.np(mybir.dt.float8e4)
    in_maps = []
    for c in range(CORES):
        shard = np.ascontiguousarray(
            x[c * ROWS:(c + 1) * ROWS].T
        ).astype(np8)
        in_maps.append({
            "xt": shard, "w0": fold["w0"], "wh": fold["wh"], "w4": fold["w4"],
        })
    return in_maps


def kernel(**inputs):
    from concourse.bass_utils import run_bass_kernel_spmd

    in_maps = _make_in_maps(inputs)
    if "nc" not in _BUILT:
        _BUILT["nc"] = _build_bass(repeat=1)
    nc = _BUILT["nc"]
    res = run_bass_kernel_spmd(nc, in_maps, core_ids=list(range(CORES)))
    _BUILT["last_results"] = res
    _BUILT["last_in_maps"] = in_maps

    o3 = np.empty((B, 3), dtype=np.float64)
    for c in range(CORES):
        o3[c * ROWS:(c + 1) * ROWS] = res.results[c]["out"].astype(np.float64).T
    o3 *= _BUILT["out_descale"]
    out = (
        _BUILT["template"][None, :, :].astype(np.float64)
        + o3[:, None, :]
    ).astype(np.float32)
    return out
